# revision 20
# baseline (speedup 1.0000x reference)
"""Trainium2 Bass kernel for nn_Attention_59339268161917.

Dense transformer attention layer (B=2, S=2048, DIM=2048, H=16, DH=128) with
RoPE, causal mask, and the reference's quirky output transpose:
    out = einsum('bhst,bhtd->bhsd', probs, v)           # [B,H,S,DH]
    out = out.transpose(0,1,3,2).reshape(B, S, DIM)     # rows = (h*DH+d), cols = s !
    y   = einsum('bsd,ed->bse', out, Wo)                # contraction over s

Sharding: 8 cores = (batch b in 0..1) x (head-group g in 0..3, 4 heads each).
Thanks to the quirky transpose, the final projection contracts over s with the
full Wo, so each core produces a DISJOINT row-slice y[b, 512g:512(g+1), :].
No collective / reduction needed; host concatenates.

Host preprocessing (= sharding-time layout choice): transposed x (xT [e,s]),
transposed+row-permuted W slices (rows deinterleaved per head: [evens; odds]
so RoPE operates on contiguous partition halves), transposed Wo, broadcast
cos/sin tables, and the 16 diagonal 128x128 mask tiles (pre-scaled by
sqrt(DH) so exp((raw + m*sqrt(DH)) / sqrt(DH)) == exp(raw/sqrt(DH) + m)).

Device pipeline per core (f32r matmuls, bf16 attention intermediates):
  A1) Q^T,K^T projections (Wq^T,Wk^T resident; xT streamed), RoPE fused into
      the PSUM->SBUF eviction on DVE.
  A2) V projection (Wv^T resident; xT streamed again).
  B)  Per (head, q-block of 128): scores matmul -> mask-add on diagonal tile
      -> exp with accumulated row-sum on ScalarE -> PE-transpose of exp'd
      probs tiles -> AV matmul accumulation -> 1/den normalize on eviction.
      Causal: strictly-upper blocks skipped (exp(-1e9)==0 exactly).
  C)  Output projection: Y[hd, e] accumulating over s-tiles, streaming Wo^T.
"""

import sys

sys.path.insert(0, "/opt/trn_rl_repo")

import numpy as np

B, S, DIM, H = 2, 2048, 2048, 16
DH = DIM // H          # 128
G = 4                  # head groups (cores per batch)
HPG = H // G           # heads per core = 4
J = HPG * DH           # per-core projection width = 512
NT = S // 128          # 16 s/t tiles
NE = DIM // 128        # 16 e tiles
SCALE = 1.0 / float(np.sqrt(DH))

_PROGRAMS = {}


def _build_program(causal: bool, phases: str = "ABC"):
    import concourse.bass as bass
    import concourse.mybir as mybir
    import concourse.tile as tile
    from concourse.masks import make_identity

    f32 = mybir.dt.float32
    f32r = mybir.dt.float32r
    bf16 = mybir.dt.bfloat16
    AF = mybir.ActivationFunctionType

    nc = bass.Bass(target_bir_lowering=False)

    # DRAM inputs (per-core shards, host-preprocessed layouts)
    xT = nc.dram_tensor("xT", [DIM, S], f32r, kind="ExternalInput")          # [e, s]
    wqT = nc.dram_tensor("wqT", [DIM, J], f32r, kind="ExternalInput")        # [e, j'] deinterleaved
    wkT = nc.dram_tensor("wkT", [DIM, J], f32r, kind="ExternalInput")
    wvT = nc.dram_tensor("wvT", [DIM, J], f32r, kind="ExternalInput")        # [e, d] original order
    woT = nc.dram_tensor("woT", [S, DIM], f32r, kind="ExternalInput")        # [s, e]
    cosb = nc.dram_tensor("cosb", [64, S], bf16, kind="ExternalInput")        # [freq, s]
    sinb = nc.dram_tensor("sinb", [64, S], bf16, kind="ExternalInput")
    # 16 diagonal 128x128 mask tiles (pre-scaled by sqrt(DH)), packed [128, 16*128]
    maskd = nc.dram_tensor("maskd", [128, NT * 128], f32, kind="ExternalInput")
    y = nc.dram_tensor("y", [J, DIM], f32, kind="ExternalOutput")            # [hd, e]

    SC = 512                   # s-chunk for phase A
    NSC = S // SC              # 4

    with tile.TileContext(nc) as tc:
        with (
            tc.tile_pool(name="const", bufs=1) as constp,
            tc.tile_pool(name="qk", bufs=1) as qkp,
        ):
            ident = constp.tile([128, 128], bf16, tag="ident")
            make_identity(nc, ident[:])

            # persistent activations (A..B): Q^T/K^T per head-tile [r;i] x s
            qT = qkp.tile([128, HPG * S], bf16, tag="qT")
            kT = qkp.tile([128, HPG * S], bf16, tag="kT")

            # =========== Phase A1: Q^T, K^T + RoPE ===========
            if "A" in phases:
             with (
                tc.tile_pool(name="a1w", bufs=1) as a1w,
                tc.tile_pool(name="a1x", bufs=2) as a1x,
                tc.tile_pool(name="rope", bufs=2) as ropep,
                tc.tile_pool(name="psA1", bufs=4, space="PSUM") as psA1,
            ):
                cos_sb = a1w.tile([64, S], bf16, tag="cos")
                sin_sb = a1w.tile([64, S], bf16, tag="sin")
                nc.sync.dma_start(cos_sb[:], cosb[:])
                nc.sync.dma_start(sin_sb[:], sinb[:])
                wq_sb = a1w.tile([128, NE, J], f32r, tag="wq")
                wk_sb = a1w.tile([128, NE, J], f32r, tag="wk")
                for eq in range(4):
                    nc.sync.dma_start(
                        wq_sb[:, eq * 4:(eq + 1) * 4, :],
                        wqT.rearrange("(ne p) j -> p ne j", p=128)[:, eq * 4:(eq + 1) * 4, :])
                for eq in range(4):
                    nc.sync.dma_start(
                        wk_sb[:, eq * 4:(eq + 1) * 4, :],
                        wkT.rearrange("(ne p) j -> p ne j", p=128)[:, eq * 4:(eq + 1) * 4, :])

                xTr = xT.rearrange("(ne p) s -> p ne s", p=128)
                xc_cache = {}
                for sc in range(NSC):
                    xc = a1x.tile([128, NE, SC], f32r, tag="xc")
                    if sc == NSC - 1:
                        xc_cache[sc] = xc
                    for eq in range(4):
                        nc.sync.dma_start(
                            xc[:, eq * 4:(eq + 1) * 4, :],
                            xTr[:, eq * 4:(eq + 1) * 4, sc * SC:(sc + 1) * SC],
                        )
                    c2 = cos_sb[:, sc * SC:(sc + 1) * SC]
                    s2 = sin_sb[:, sc * SC:(sc + 1) * SC]
                    for w_sb, dstT in ((wq_sb, qT), (wk_sb, kT)):
                        for jt in range(HPG):
                            ps = psA1.tile([128, SC], f32, tag="ps_qk")
                            for et in range(NE):
                                nc.tensor.matmul(
                                    ps[:],
                                    w_sb[:, et, jt * 128:(jt + 1) * 128],
                                    xc[:, et, :],
                                    start=(et == 0), stop=(et == NE - 1),
                                )
                            # RoPE: rows 0:64 = r (even feats), 64:128 = i (odd)
                            dst = dstT[:, jt * S + sc * SC: jt * S + (sc + 1) * SC]
                            t1 = ropep.tile([64, SC], bf16, tag="t1")
                            t2 = ropep.tile([64, SC], bf16, tag="t2")
                            nc.vector.tensor_mul(t1[:], ps[:64, :], c2)
                            nc.vector.tensor_mul(t2[:], ps[64:, :], s2)
                            nc.gpsimd.tensor_sub(dst[:64, :], t1[:], t2[:])
                            t3 = ropep.tile([64, SC], bf16, tag="t1")
                            t4 = ropep.tile([64, SC], bf16, tag="t2")
                            nc.vector.tensor_mul(t3[:], ps[:64, :], s2)
                            nc.vector.tensor_mul(t4[:], ps[64:, :], c2)
                            nc.gpsimd.tensor_add(dst[64:, :], t3[:], t4[:])

                # ---- V projection (same scope; wv reuses the wq slot) ----
                vN = qkp.tile([128, NT * J], bf16, tag="vN")   # V natural [t, d]
                wv_sb = a1w.tile([128, NE, J], f32r, tag="wq")
                nc.sync.dma_start(wv_sb[:], wvT.rearrange("(ne p) j -> p ne j", p=128))
                for sc in [NSC - 1] + list(range(NSC - 1)):
                    if sc in xc_cache:
                        xc = xc_cache[sc]
                    else:
                        xc = a1x.tile([128, NE, SC], f32r, tag="xc")
                        for eq in range(4):
                            nc.sync.dma_start(
                                xc[:, eq * 4:(eq + 1) * 4, :],
                                xTr[:, eq * 4:(eq + 1) * 4, sc * SC:(sc + 1) * SC],
                            )
                    for tt in range(SC // 128):
                        ps = psA1.tile([128, J], f32, tag="ps_qk")
                        for et in range(NE):
                            nc.tensor.matmul(
                                ps[:],
                                xc[:, et, tt * 128:(tt + 1) * 128],
                                wv_sb[:, et, :],
                                start=(et == 0), stop=(et == NE - 1),
                            )
                        gt = sc * (SC // 128) + tt
                        nc.scalar.activation(vN[:, gt * J:(gt + 1) * J], ps[:], AF.Copy)

            # =========== Phase B: attention ===========
            with (
                tc.tile_pool(name="oN", bufs=1) as onp,
                tc.tile_pool(name="att", bufs=3) as attp,
                tc.tile_pool(name="psS", bufs=2, space="PSUM") as psS,
                tc.tile_pool(name="psT", bufs=2, space="PSUM") as psT,
                tc.tile_pool(name="psO", bufs=2, space="PSUM") as psO,
            ):
                # O natural: s-tile st -> [:, st*J:(st+1)*J] = [128 s, J hd]
                oN = onp.tile([128, NT * J], f32r, tag="oN")

                for ht in range(HPG):
                    qh = qT[:, ht * S:(ht + 1) * S]
                    kh = kT[:, ht * S:(ht + 1) * S]
                    for qb in range(NT):
                        t_ext = (qb + 1) * 128 if causal else S
                        nkt = t_ext // 128
                        nch = (t_ext + 511) // 512
                        e_sb = attp.tile([128, S], bf16, tag="e_sb")
                        den_parts = attp.tile([128, 4], f32, tag="denp")
                        for ch in range(nch):
                            n = min(512, t_ext - ch * 512)
                            ps = psS.tile([128, 512], f32, tag="ps_s")
                            nc.tensor.matmul(
                                ps[:, :n],
                                qh[:, qb * 128:(qb + 1) * 128],
                                kh[:, ch * 512: ch * 512 + n],
                                start=True, stop=True,
                            )
                            if causal and ch == nch - 1:
                                nc.vector.tensor_add(
                                    ps[:, n - 128:n], ps[:, n - 128:n],
                                    mask_sb[:, qb * 128:(qb + 1) * 128],
                                )
                            nc.scalar.activation(
                                e_sb[:, ch * 512: ch * 512 + n], ps[:, :n], AF.Exp,
                                scale=SCALE, accum_out=den_parts[:, ch:ch + 1],
                            )
                        den = attp.tile([128, 1], f32, tag="den")
                        if nch == 1:
                            rec_src = den_parts[:, 0:1]
                        else:
                            nc.vector.tensor_add(den[:], den_parts[:, 0:1], den_parts[:, 1:2])
                            for ch in range(2, nch):
                                nc.vector.tensor_add(den[:], den[:], den_parts[:, ch:ch + 1])
                            rec_src = den[:]
                        rec = attp.tile([128, 1], f32, tag="rec")
                        nc.vector.reciprocal(rec[:], rec_src)

                        eT_sb = attp.tile([128, NT * 128], bf16, tag="eT")
                        for kt in range(nkt):
                            pt = psT.tile([128, 128], bf16, tag="ps_t")
                            nc.tensor.transpose(pt[:], e_sb[:, kt * 128:(kt + 1) * 128], ident[:])
                            dst = eT_sb[:, kt * 128:(kt + 1) * 128]
                            if kt % 2 == 0:
                                nc.vector.tensor_copy(dst, pt[:])
                            else:
                                nc.scalar.activation(dst, pt[:], AF.Copy)
                        po = psO.tile([128, 128], f32, tag="ps_o")
                        for kt in range(nkt):
                            nc.tensor.matmul(
                                po[:],
                                eT_sb[:, kt * 128:(kt + 1) * 128],
                                vN[:, kt * J + ht * 128: kt * J + (ht + 1) * 128],
                                start=(kt == 0), stop=(kt == nkt - 1),
                            )
                        nc.scalar.activation(
                            oN[:, qb * J + ht * 128: qb * J + (ht + 1) * 128],
                            po[:], AF.Copy, scale=rec[:],
                        )

                # =========== Phase C: output projection ===========
                EC = 512
                if "C" in phases:
                 with (
                    tc.tile_pool(name="wo", bufs=2) as wop,
                    tc.tile_pool(name="psY", bufs=4, space="PSUM") as psY,
                ):
                    for ec in range(DIM // EC):
                        woc = wop.tile([128, NT, EC], f32r, tag="woc")
                        woTr = woT.rearrange("(nt p) e -> p nt e", p=128)
                        for sq in range(4):
                            nc.sync.dma_start(
                                woc[:, sq * 4:(sq + 1) * 4, :],
                                woTr[:, sq * 4:(sq + 1) * 4, ec * EC:(ec + 1) * EC],
                            )
                        for jt in range(HPG):
                            ps = psY.tile([128, EC], f32, tag="ps_y")
                            for st in range(NT):
                                nc.tensor.matmul(
                                    ps[:],
                                    oN[:, st * J + jt * 128: st * J + (jt + 1) * 128],
                                    woc[:, st, :],
                                    start=(st == 0), stop=(st == NT - 1),
                                )
                            ysb = wop.tile([128, EC], f32, tag="ysb")
                            nc.scalar.activation(ysb[:], ps[:], AF.Copy)
                            nc.sync.dma_start(
                                y[jt * 128:(jt + 1) * 128, ec * EC:(ec + 1) * EC], ysb[:]
                            )

    import bass_rust
    bass_rust.move_matmul_waits_to_ldweights(nc.m)
    bass_rust.generate_event_semaphores(nc)
    return nc


def _get_program(causal: bool):
    if causal not in _PROGRAMS:
        _PROGRAMS[causal] = _build_program(causal)
    return _PROGRAMS[causal]


def _deinterleave_rows(w_slice):
    """Permute [128k, E] rows within each 128-row head block: evens then odds."""
    out = w_slice.reshape(-1, DH, w_slice.shape[-1])
    return np.concatenate([out[:, 0::2, :], out[:, 1::2, :]], axis=1).reshape(w_slice.shape)


def _is_causal_compatible(mask2d):
    causal_ref = np.triu(np.full((S, S), -1e9, dtype=np.float32), k=1)
    if np.array_equal(mask2d, causal_ref):
        return True
    # any mask that is 0 on/below the block sub-diagonal region outside the
    # diagonal tiles and <= -1e8 strictly above the diagonal tiles also works
    for i in range(NT):
        lo = mask2d[i * 128:(i + 1) * 128, : i * 128]
        if lo.size and not np.all(lo == 0.0):
            return False
        up = mask2d[i * 128:(i + 1) * 128, (i + 1) * 128:]
        if up.size and not np.all(up <= -1e8):
            return False
    return True


def _make_in_maps(inputs):
    x = np.asarray(inputs["x"], dtype=np.float32)
    Wq = np.asarray(inputs["Wq"], dtype=np.float32)
    Wk = np.asarray(inputs["Wk"], dtype=np.float32)
    Wv = np.asarray(inputs["Wv"], dtype=np.float32)
    Wo = np.asarray(inputs["Wo"], dtype=np.float32)
    freqs_cos = np.asarray(inputs["freqs_cos"], dtype=np.float32)
    freqs_sin = np.asarray(inputs["freqs_sin"], dtype=np.float32)
    mask2d = np.asarray(inputs["mask"], dtype=np.float32).reshape(S, S)

    import ml_dtypes
    cosb = np.ascontiguousarray(freqs_cos.T).astype(ml_dtypes.bfloat16)
    sinb = np.ascontiguousarray(freqs_sin.T).astype(ml_dtypes.bfloat16)
    maskd = np.concatenate(
        [mask2d[i * 128:(i + 1) * 128, i * 128:(i + 1) * 128] for i in range(NT)], axis=1
    ) * np.float32(np.sqrt(DH))
    maskd = np.ascontiguousarray(maskd, dtype=np.float32)
    woT = np.ascontiguousarray(Wo.T)

    in_maps = []
    for c in range(8):
        b, g = divmod(c, G)
        rows = slice(g * J, (g + 1) * J)
        in_maps.append({
            "xT": np.ascontiguousarray(x[b].T),
            "wqT": np.ascontiguousarray(_deinterleave_rows(Wq[rows]).T),
            "wkT": np.ascontiguousarray(_deinterleave_rows(Wk[rows]).T),
            "wvT": np.ascontiguousarray(Wv[rows].T),
            "woT": woT,
            "cosb": cosb,
            "sinb": sinb,
            "maskd": maskd,
        })
    return in_maps


def _offdiag_tiles_zero(mask2d):
    m = mask2d.copy()
    for i in range(NT):
        m[i * 128:(i + 1) * 128, i * 128:(i + 1) * 128] = 0.0
    return bool(np.all(m == 0.0))


def _numpy_fallback(x, Wq, Wk, Wv, Wo, freqs_cos, freqs_sin, mask):
    q = (x @ Wq.T).reshape(B, S, H, DH)
    k = (x @ Wk.T).reshape(B, S, H, DH)
    v = (x @ Wv.T).reshape(B, S, H, DH)

    def rope(t):
        tr, ti = t[..., 0::2], t[..., 1::2]
        c = freqs_cos[None, :, None, :]
        s = freqs_sin[None, :, None, :]
        return np.stack([tr * c - ti * s, tr * s + ti * c], axis=-1).reshape(t.shape)

    q, k = rope(q), rope(k)
    q, k, v = (t.transpose(0, 2, 1, 3) for t in (q, k, v))
    m = mask.reshape(S, S)
    out = np.empty((B, H, S, DH), np.float32)
    for b in range(B):
        for h in range(H):
            sc = (q[b, h] @ k[b, h].T) / np.float32(np.sqrt(DH)) + m
            sc -= sc.max(axis=1, keepdims=True)
            e = np.exp(sc)
            out[b, h] = (e / e.sum(axis=1, keepdims=True)) @ v[b, h]
    out = out.transpose(0, 1, 3, 2).reshape(B, S, DIM)
    return (out @ Wo.T).astype(np.float32)


def kernel(x, Wq, Wk, Wv, Wo, freqs_cos, freqs_sin, mask):
    from concourse.bass_utils import run_bass_kernel_spmd

    inputs = {"x": x, "Wq": Wq, "Wk": Wk, "Wv": Wv, "Wo": Wo,
              "freqs_cos": freqs_cos, "freqs_sin": freqs_sin, "mask": mask}
    mask2d = np.asarray(mask, dtype=np.float32).reshape(S, S)
    causal = _is_causal_compatible(mask2d)
    if not causal and not _offdiag_tiles_zero(mask2d):
        return _numpy_fallback(
            np.asarray(x, np.float32), np.asarray(Wq, np.float32),
            np.asarray(Wk, np.float32), np.asarray(Wv, np.float32),
            np.asarray(Wo, np.float32), np.asarray(freqs_cos, np.float32),
            np.asarray(freqs_sin, np.float32), mask2d)
    nc = _get_program(causal)
    in_maps = _make_in_maps(inputs)

    res = run_bass_kernel_spmd(nc, in_maps, core_ids=list(range(8)))

    out = np.empty((B, S, DIM), dtype=np.float32)
    for c in range(8):
        b, g = divmod(c, G)
        out[b, g * J:(g + 1) * J, :] = res.results[c]["y"]
    return out


# revision 24
# speedup vs baseline: 1.0039x; 1.0039x over previous
"""Trainium2 Bass kernel for nn_Attention_59339268161917.

Dense transformer attention layer (B=2, S=2048, DIM=2048, H=16, DH=128) with
RoPE, causal mask, and the reference's quirky output transpose:
    out = einsum('bhst,bhtd->bhsd', probs, v)           # [B,H,S,DH]
    out = out.transpose(0,1,3,2).reshape(B, S, DIM)     # rows = (h*DH+d), cols = s !
    y   = einsum('bsd,ed->bse', out, Wo)                # contraction over s

Sharding: 8 cores = (batch b in 0..1) x (head-group g in 0..3, 4 heads each).
Thanks to the quirky transpose, the final projection contracts over s with the
full Wo, so each core produces a DISJOINT row-slice y[b, 512g:512(g+1), :].
No collective / reduction needed; host concatenates.

Host preprocessing (= sharding-time layout choice): transposed x (xT [e,s]),
transposed+row-permuted W slices (rows deinterleaved per head: [evens; odds]
so RoPE operates on contiguous partition halves), transposed Wo, broadcast
cos/sin tables, and the 16 diagonal 128x128 mask tiles (pre-scaled by
sqrt(DH) so exp((raw + m*sqrt(DH)) / sqrt(DH)) == exp(raw/sqrt(DH) + m)).

Device pipeline per core (f32r matmuls, bf16 attention intermediates):
  A1) Q^T,K^T projections (Wq^T,Wk^T resident; xT streamed), RoPE fused into
      the PSUM->SBUF eviction on DVE.
  A2) V projection (Wv^T resident; xT streamed again).
  B)  Per (head, q-block of 128): scores matmul -> mask-add on diagonal tile
      -> exp with accumulated row-sum on ScalarE -> PE-transpose of exp'd
      probs tiles -> AV matmul accumulation -> 1/den normalize on eviction.
      Causal: strictly-upper blocks skipped (exp(-1e9)==0 exactly).
  C)  Output projection: Y[hd, e] accumulating over s-tiles, streaming Wo^T.
"""

import sys

sys.path.insert(0, "/opt/trn_rl_repo")

import numpy as np

B, S, DIM, H = 2, 2048, 2048, 16
DH = DIM // H          # 128
G = 4                  # head groups (cores per batch)
HPG = H // G           # heads per core = 4
J = HPG * DH           # per-core projection width = 512
NT = S // 128          # 16 s/t tiles
NE = DIM // 128        # 16 e tiles
SCALE = 1.0 / float(np.sqrt(DH))

_PROGRAMS = {}


def _build_program(causal: bool, phases: str = "ABC"):
    import concourse.bass as bass
    import concourse.mybir as mybir
    import concourse.tile as tile
    from concourse.masks import make_identity

    f32 = mybir.dt.float32
    f32r = mybir.dt.float32r
    bf16 = mybir.dt.bfloat16
    AF = mybir.ActivationFunctionType

    nc = bass.Bass(target_bir_lowering=False)

    # DRAM inputs (per-core shards, host-preprocessed layouts)
    xT = nc.dram_tensor("xT", [DIM, S], f32r, kind="ExternalInput")          # [e, s]
    wqT = nc.dram_tensor("wqT", [DIM, J], f32r, kind="ExternalInput")        # [e, j'] deinterleaved
    wkT = nc.dram_tensor("wkT", [DIM, J], f32r, kind="ExternalInput")
    wvT = nc.dram_tensor("wvT", [DIM, J], f32r, kind="ExternalInput")        # [e, d] original order
    woT = nc.dram_tensor("woT", [S, DIM], f32r, kind="ExternalInput")        # [s, e]
    cosb = nc.dram_tensor("cosb", [64, S], bf16, kind="ExternalInput")        # [freq, s]
    sinb = nc.dram_tensor("sinb", [64, S], bf16, kind="ExternalInput")
    # 16 diagonal 128x128 mask tiles (pre-scaled by sqrt(DH)), packed [128, 16*128]
    maskd = nc.dram_tensor("maskd", [128, NT * 128], bf16, kind="ExternalInput")
    y = nc.dram_tensor("y", [J, DIM], f32, kind="ExternalOutput")            # [hd, e]

    SC = 512                   # s-chunk for phase A
    NSC = S // SC              # 4

    with tile.TileContext(nc) as tc:
        with (
            tc.tile_pool(name="const", bufs=1) as constp,
            tc.tile_pool(name="qk", bufs=1) as qkp,
        ):
            ident = constp.tile([128, 128], bf16, tag="ident")
            make_identity(nc, ident[:])

            # persistent activations (A..B): Q^T/K^T per head-tile [r;i] x s
            qT = qkp.tile([128, HPG * S], bf16, tag="qT")
            kT = qkp.tile([128, HPG * S], bf16, tag="kT")
            mask_sb = constp.tile([128, NT * 128], bf16, tag="mask")
            nc.sync.dma_start(mask_sb[:], maskd[:])

            # =========== Phase A1: Q^T, K^T + RoPE ===========
            if "A" in phases:
             with (
                tc.tile_pool(name="a1w", bufs=1) as a1w,
                tc.tile_pool(name="a1x", bufs=2) as a1x,
                tc.tile_pool(name="rope", bufs=2) as ropep,
                tc.tile_pool(name="psA1", bufs=6, space="PSUM") as psA1,
            ):
                cos_sb = a1w.tile([64, S], bf16, tag="cos")
                sin_sb = a1w.tile([64, S], bf16, tag="sin")
                nc.sync.dma_start(cos_sb[:], cosb[:])
                nc.sync.dma_start(sin_sb[:], sinb[:])
                wq_sb = a1w.tile([128, NE, J], f32r, tag="wq")
                wk_sb = a1w.tile([128, NE, J], f32r, tag="wk")
                for eq in range(4):
                    nc.sync.dma_start(
                        wq_sb[:, eq * 4:(eq + 1) * 4, :],
                        wqT.rearrange("(ne p) j -> p ne j", p=128)[:, eq * 4:(eq + 1) * 4, :])
                for eq in range(4):
                    nc.sync.dma_start(
                        wk_sb[:, eq * 4:(eq + 1) * 4, :],
                        wkT.rearrange("(ne p) j -> p ne j", p=128)[:, eq * 4:(eq + 1) * 4, :])

                xTr = xT.rearrange("(ne p) s -> p ne s", p=128)
                xc_cache = {}
                for sc in range(NSC):
                    xc = a1x.tile([128, NE, SC], f32r, tag="xc")
                    if sc == NSC - 1:
                        xc_cache[sc] = xc
                    for eq in range(4):
                        nc.sync.dma_start(
                            xc[:, eq * 4:(eq + 1) * 4, :],
                            xTr[:, eq * 4:(eq + 1) * 4, sc * SC:(sc + 1) * SC],
                        )
                    c2 = cos_sb[:, sc * SC:(sc + 1) * SC]
                    s2 = sin_sb[:, sc * SC:(sc + 1) * SC]
                    for w_sb, dstT in ((wq_sb, qT), (wk_sb, kT)):
                        for jt in range(HPG):
                            ps = psA1.tile([128, SC], f32, tag="ps_qk")
                            for et in range(NE):
                                nc.tensor.matmul(
                                    ps[:],
                                    w_sb[:, et, jt * 128:(jt + 1) * 128],
                                    xc[:, et, :],
                                    start=(et == 0), stop=(et == NE - 1),
                                )
                            # RoPE: rows 0:64 = r (even feats), 64:128 = i (odd)
                            dst = dstT[:, jt * S + sc * SC: jt * S + (sc + 1) * SC]
                            t1 = ropep.tile([64, SC], bf16, tag="t1")
                            t2 = ropep.tile([64, SC], bf16, tag="t2")
                            nc.vector.tensor_mul(t1[:], ps[:64, :], c2)
                            nc.vector.tensor_mul(t2[:], ps[64:, :], s2)
                            nc.gpsimd.tensor_sub(dst[:64, :], t1[:], t2[:])
                            t3 = ropep.tile([64, SC], bf16, tag="t1")
                            t4 = ropep.tile([64, SC], bf16, tag="t2")
                            nc.vector.tensor_mul(t3[:], ps[:64, :], s2)
                            nc.vector.tensor_mul(t4[:], ps[64:, :], c2)
                            nc.gpsimd.tensor_add(dst[64:, :], t3[:], t4[:])

                # ---- V projection (same scope; wv reuses the wq slot) ----
                vN = qkp.tile([128, NT * J], bf16, tag="vN")   # V natural [t, d]
                wv_sb = a1w.tile([128, NE, J], f32r, tag="wq")
                nc.sync.dma_start(wv_sb[:], wvT.rearrange("(ne p) j -> p ne j", p=128))
                for sc in [NSC - 1] + list(range(NSC - 1)):
                    if sc in xc_cache:
                        xc = xc_cache[sc]
                    else:
                        xc = a1x.tile([128, NE, SC], f32r, tag="xc")
                        for eq in range(4):
                            nc.sync.dma_start(
                                xc[:, eq * 4:(eq + 1) * 4, :],
                                xTr[:, eq * 4:(eq + 1) * 4, sc * SC:(sc + 1) * SC],
                            )
                    for tt in range(SC // 128):
                        ps = psA1.tile([128, J], f32, tag="ps_qk")
                        for et in range(NE):
                            nc.tensor.matmul(
                                ps[:],
                                xc[:, et, tt * 128:(tt + 1) * 128],
                                wv_sb[:, et, :],
                                start=(et == 0), stop=(et == NE - 1),
                            )
                        gt = sc * (SC // 128) + tt
                        nc.scalar.activation(vN[:, gt * J:(gt + 1) * J], ps[:], AF.Copy)

            # =========== Phase B: attention ===========
            with (
                tc.tile_pool(name="oN", bufs=1) as onp,
                tc.tile_pool(name="att", bufs=3) as attp,
                tc.tile_pool(name="psS", bufs=2, space="PSUM") as psS,
                tc.tile_pool(name="psT", bufs=2, space="PSUM") as psT,
                tc.tile_pool(name="psO", bufs=2, space="PSUM") as psO,
            ):
                # O natural: s-tile st -> [:, st*J:(st+1)*J] = [128 s, J hd]
                oN = onp.tile([128, NT * J], f32r, tag="oN")

                for ht in range(HPG):
                    qh = qT[:, ht * S:(ht + 1) * S]
                    kh = kT[:, ht * S:(ht + 1) * S]
                    for qb in range(NT):
                        t_ext = (qb + 1) * 128 if causal else S
                        nkt = t_ext // 128
                        nch = (t_ext + 511) // 512
                        e_sb = attp.tile([128, S], bf16, tag="e_sb")
                        den_parts = attp.tile([128, 4], f32, tag="denp")
                        for ch in range(nch):
                            n = min(512, t_ext - ch * 512)
                            ps = psS.tile([128, 512], f32, tag="ps_s")
                            nc.tensor.matmul(
                                ps[:, :n],
                                qh[:, qb * 128:(qb + 1) * 128],
                                kh[:, ch * 512: ch * 512 + n],
                                start=True, stop=True,
                            )
                            if causal and ch == nch - 1:
                                nc.vector.tensor_add(
                                    ps[:, n - 128:n], ps[:, n - 128:n],
                                    mask_sb[:, qb * 128:(qb + 1) * 128],
                                )
                            nc.scalar.activation(
                                e_sb[:, ch * 512: ch * 512 + n], ps[:, :n], AF.Exp,
                                scale=SCALE, accum_out=den_parts[:, ch:ch + 1],
                            )
                        den = attp.tile([128, 1], f32, tag="den")
                        if nch == 1:
                            rec_src = den_parts[:, 0:1]
                        else:
                            nc.vector.tensor_add(den[:], den_parts[:, 0:1], den_parts[:, 1:2])
                            for ch in range(2, nch):
                                nc.vector.tensor_add(den[:], den[:], den_parts[:, ch:ch + 1])
                            rec_src = den[:]
                        rec = attp.tile([128, 1], f32, tag="rec")
                        nc.vector.reciprocal(rec[:], rec_src)

                        eT_sb = attp.tile([128, NT * 128], bf16, tag="eT")
                        for kt in range(nkt):
                            pt = psT.tile([128, 128], bf16, tag="ps_t")
                            nc.tensor.transpose(pt[:], e_sb[:, kt * 128:(kt + 1) * 128], ident[:])
                            dst = eT_sb[:, kt * 128:(kt + 1) * 128]
                            if kt % 2 == 0:
                                nc.vector.tensor_copy(dst, pt[:])
                            else:
                                nc.scalar.activation(dst, pt[:], AF.Copy)
                        po = psO.tile([128, 128], f32, tag="ps_o")
                        for kt in range(nkt):
                            nc.tensor.matmul(
                                po[:],
                                eT_sb[:, kt * 128:(kt + 1) * 128],
                                vN[:, kt * J + ht * 128: kt * J + (ht + 1) * 128],
                                start=(kt == 0), stop=(kt == nkt - 1),
                            )
                        nc.scalar.activation(
                            oN[:, qb * J + ht * 128: qb * J + (ht + 1) * 128],
                            po[:], AF.Copy, scale=rec[:],
                        )

                # =========== Phase C: output projection ===========
                EC = 512
                if "C" in phases:
                 with (
                    tc.tile_pool(name="wo", bufs=2) as wop,
                    tc.tile_pool(name="psY", bufs=4, space="PSUM") as psY,
                ):
                    for ec in range(DIM // EC):
                        woc = wop.tile([128, NT, EC], f32r, tag="woc")
                        woTr = woT.rearrange("(nt p) e -> p nt e", p=128)
                        for sq in range(4):
                            nc.sync.dma_start(
                                woc[:, sq * 4:(sq + 1) * 4, :],
                                woTr[:, sq * 4:(sq + 1) * 4, ec * EC:(ec + 1) * EC],
                            )
                        for jt in range(HPG):
                            ps = psY.tile([128, EC], f32, tag="ps_y")
                            for st in range(NT):
                                nc.tensor.matmul(
                                    ps[:],
                                    oN[:, st * J + jt * 128: st * J + (jt + 1) * 128],
                                    woc[:, st, :],
                                    start=(st == 0), stop=(st == NT - 1),
                                )
                            ysb = wop.tile([128, EC], f32, tag="ysb")
                            nc.scalar.activation(ysb[:], ps[:], AF.Copy)
                            nc.sync.dma_start(
                                y[jt * 128:(jt + 1) * 128, ec * EC:(ec + 1) * EC], ysb[:]
                            )

    import bass_rust
    bass_rust.move_matmul_waits_to_ldweights(nc.m)
    bass_rust.generate_event_semaphores(nc)
    return nc


def _get_program(causal: bool):
    if causal not in _PROGRAMS:
        _PROGRAMS[causal] = _build_program(causal)
    return _PROGRAMS[causal]


def _deinterleave_rows(w_slice):
    """Permute [128k, E] rows within each 128-row head block: evens then odds."""
    out = w_slice.reshape(-1, DH, w_slice.shape[-1])
    return np.concatenate([out[:, 0::2, :], out[:, 1::2, :]], axis=1).reshape(w_slice.shape)


def _is_causal_compatible(mask2d):
    causal_ref = np.triu(np.full((S, S), -1e9, dtype=np.float32), k=1)
    if np.array_equal(mask2d, causal_ref):
        return True
    # any mask that is 0 on/below the block sub-diagonal region outside the
    # diagonal tiles and <= -1e8 strictly above the diagonal tiles also works
    for i in range(NT):
        lo = mask2d[i * 128:(i + 1) * 128, : i * 128]
        if lo.size and not np.all(lo == 0.0):
            return False
        up = mask2d[i * 128:(i + 1) * 128, (i + 1) * 128:]
        if up.size and not np.all(up <= -1e8):
            return False
    return True


def _make_in_maps(inputs):
    x = np.asarray(inputs["x"], dtype=np.float32)
    Wq = np.asarray(inputs["Wq"], dtype=np.float32)
    Wk = np.asarray(inputs["Wk"], dtype=np.float32)
    Wv = np.asarray(inputs["Wv"], dtype=np.float32)
    Wo = np.asarray(inputs["Wo"], dtype=np.float32)
    freqs_cos = np.asarray(inputs["freqs_cos"], dtype=np.float32)
    freqs_sin = np.asarray(inputs["freqs_sin"], dtype=np.float32)
    mask2d = np.asarray(inputs["mask"], dtype=np.float32).reshape(S, S)

    import ml_dtypes
    cosb = np.ascontiguousarray(freqs_cos.T).astype(ml_dtypes.bfloat16)
    sinb = np.ascontiguousarray(freqs_sin.T).astype(ml_dtypes.bfloat16)
    maskd = np.concatenate(
        [mask2d[i * 128:(i + 1) * 128, i * 128:(i + 1) * 128] for i in range(NT)], axis=1
    ) * np.float32(np.sqrt(DH))
    import ml_dtypes as _mld
    maskd = np.ascontiguousarray(maskd).astype(_mld.bfloat16)
    woT = np.ascontiguousarray(Wo.T)

    in_maps = []
    for c in range(8):
        b, g = divmod(c, G)
        rows = slice(g * J, (g + 1) * J)
        in_maps.append({
            "xT": np.ascontiguousarray(x[b].T),
            "wqT": np.ascontiguousarray(_deinterleave_rows(Wq[rows]).T),
            "wkT": np.ascontiguousarray(_deinterleave_rows(Wk[rows]).T),
            "wvT": np.ascontiguousarray(Wv[rows].T),
            "woT": woT,
            "cosb": cosb,
            "sinb": sinb,
            "maskd": maskd,
        })
    return in_maps


def _offdiag_tiles_zero(mask2d):
    m = mask2d.copy()
    for i in range(NT):
        m[i * 128:(i + 1) * 128, i * 128:(i + 1) * 128] = 0.0
    return bool(np.all(m == 0.0))


def _numpy_fallback(x, Wq, Wk, Wv, Wo, freqs_cos, freqs_sin, mask):
    q = (x @ Wq.T).reshape(B, S, H, DH)
    k = (x @ Wk.T).reshape(B, S, H, DH)
    v = (x @ Wv.T).reshape(B, S, H, DH)

    def rope(t):
        tr, ti = t[..., 0::2], t[..., 1::2]
        c = freqs_cos[None, :, None, :]
        s = freqs_sin[None, :, None, :]
        return np.stack([tr * c - ti * s, tr * s + ti * c], axis=-1).reshape(t.shape)

    q, k = rope(q), rope(k)
    q, k, v = (t.transpose(0, 2, 1, 3) for t in (q, k, v))
    m = mask.reshape(S, S)
    out = np.empty((B, H, S, DH), np.float32)
    for b in range(B):
        for h in range(H):
            sc = (q[b, h] @ k[b, h].T) / np.float32(np.sqrt(DH)) + m
            sc -= sc.max(axis=1, keepdims=True)
            e = np.exp(sc)
            out[b, h] = (e / e.sum(axis=1, keepdims=True)) @ v[b, h]
    out = out.transpose(0, 1, 3, 2).reshape(B, S, DIM)
    return (out @ Wo.T).astype(np.float32)


def kernel(x, Wq, Wk, Wv, Wo, freqs_cos, freqs_sin, mask):
    from concourse.bass_utils import run_bass_kernel_spmd

    inputs = {"x": x, "Wq": Wq, "Wk": Wk, "Wv": Wv, "Wo": Wo,
              "freqs_cos": freqs_cos, "freqs_sin": freqs_sin, "mask": mask}
    mask2d = np.asarray(mask, dtype=np.float32).reshape(S, S)
    causal = _is_causal_compatible(mask2d)
    if not causal and not _offdiag_tiles_zero(mask2d):
        return _numpy_fallback(
            np.asarray(x, np.float32), np.asarray(Wq, np.float32),
            np.asarray(Wk, np.float32), np.asarray(Wv, np.float32),
            np.asarray(Wo, np.float32), np.asarray(freqs_cos, np.float32),
            np.asarray(freqs_sin, np.float32), mask2d)
    nc = _get_program(causal)
    in_maps = _make_in_maps(inputs)

    res = run_bass_kernel_spmd(nc, in_maps, core_ids=list(range(8)))

    out = np.empty((B, S, DIM), dtype=np.float32)
    for c in range(8):
        b, g = divmod(c, G)
        out[b, g * J:(g + 1) * J, :] = res.results[c]["y"]
    return out


# revision 28
# speedup vs baseline: 1.0715x; 1.0672x over previous
"""Trainium2 Bass kernel for nn_Attention_59339268161917.

Dense transformer attention layer (B=2, S=2048, DIM=2048, H=16, DH=128) with
RoPE, causal mask, and the reference's quirky output transpose:
    out = einsum('bhst,bhtd->bhsd', probs, v)           # [B,H,S,DH]
    out = out.transpose(0,1,3,2).reshape(B, S, DIM)     # rows = (h*DH+d), cols = s !
    y   = einsum('bsd,ed->bse', out, Wo)                # contraction over s

Sharding: 8 cores = (batch b in 0..1) x (head-group g in 0..3, 4 heads each).
Thanks to the quirky transpose, the final projection contracts over s with the
full Wo, so each core produces a DISJOINT row-slice y[b, 512g:512(g+1), :].
No collective / reduction needed; host concatenates.

Host preprocessing (= sharding-time layout choice): transposed x (xT [e,s]),
transposed+row-permuted W slices (rows deinterleaved per head: [evens; odds]
so RoPE operates on contiguous partition halves), transposed Wo, broadcast
cos/sin tables, and the 16 diagonal 128x128 mask tiles (pre-scaled by
sqrt(DH) so exp((raw + m*sqrt(DH)) / sqrt(DH)) == exp(raw/sqrt(DH) + m)).

Device pipeline per core (f32r matmuls, bf16 attention intermediates):
  A1) Q^T,K^T projections (Wq^T,Wk^T resident; xT streamed), RoPE fused into
      the PSUM->SBUF eviction on DVE.
  A2) V projection (Wv^T resident; xT streamed again).
  B)  Per (head, q-block of 128): scores matmul -> mask-add on diagonal tile
      -> exp with accumulated row-sum on ScalarE -> PE-transpose of exp'd
      probs tiles -> AV matmul accumulation -> 1/den normalize on eviction.
      Causal: strictly-upper blocks skipped (exp(-1e9)==0 exactly).
  C)  Output projection: Y[hd, e] accumulating over s-tiles, streaming Wo^T.
"""

import sys

sys.path.insert(0, "/opt/trn_rl_repo")

import numpy as np

B, S, DIM, H = 2, 2048, 2048, 16
DH = DIM // H          # 128
G = 4                  # head groups (cores per batch)
HPG = H // G           # heads per core = 4
J = HPG * DH           # per-core projection width = 512
NT = S // 128          # 16 s/t tiles
NE = DIM // 128        # 16 e tiles
SCALE = 1.0 / float(np.sqrt(DH))

_PROGRAMS = {}


def _build_program(causal: bool, phases: str = "ABC"):
    import concourse.bass as bass
    import concourse.mybir as mybir
    import concourse.tile as tile
    from concourse.masks import make_identity

    f32 = mybir.dt.float32
    f32r = mybir.dt.float32r
    bf16 = mybir.dt.bfloat16
    AF = mybir.ActivationFunctionType

    nc = bass.Bass(target_bir_lowering=False)

    # DRAM inputs (per-core shards, host-preprocessed layouts)
    xT = nc.dram_tensor("xT", [DIM, S], f32r, kind="ExternalInput")          # [e, s]
    wqT = nc.dram_tensor("wqT", [DIM, J], f32r, kind="ExternalInput")        # [e, j'] deinterleaved
    wkT = nc.dram_tensor("wkT", [DIM, J], f32r, kind="ExternalInput")
    wvT = nc.dram_tensor("wvT", [DIM, J], f32r, kind="ExternalInput")        # [e, d] original order
    woT = nc.dram_tensor("woT", [S, DIM], f32r, kind="ExternalInput")        # [s, e]
    cosb = nc.dram_tensor("cosb", [64, S], bf16, kind="ExternalInput")        # [freq, s]
    sinb = nc.dram_tensor("sinb", [64, S], bf16, kind="ExternalInput")
    # 16 diagonal 128x128 mask tiles (pre-scaled by sqrt(DH)), packed [128, 16*128]
    maskd = nc.dram_tensor("maskd", [128, NT * 128], bf16, kind="ExternalInput")
    y = nc.dram_tensor("y", [J, DIM], f32, kind="ExternalOutput")            # [hd, e]

    SC = 512                   # s-chunk for phase A
    NSC = S // SC              # 4

    with tile.TileContext(nc) as tc:
        with (
            tc.tile_pool(name="const", bufs=1) as constp,
            tc.tile_pool(name="qk", bufs=1) as qkp,
        ):
            ident = constp.tile([128, 128], bf16, tag="ident")
            make_identity(nc, ident[:])

            # persistent activations (A..B): Q^T/K^T per head-tile [r;i] x s
            qT = qkp.tile([128, HPG * S], bf16, tag="qT")
            kT = qkp.tile([128, HPG * S], bf16, tag="kT")
            mask_sb = constp.tile([128, NT * 128], bf16, tag="mask")

            # =========== Phase A1: Q^T, K^T + RoPE ===========
            if "A" in phases:
             with (
                tc.tile_pool(name="a1w", bufs=1) as a1w,
                tc.tile_pool(name="a1x", bufs=2) as a1x,
                tc.tile_pool(name="rope", bufs=2) as ropep,
                tc.tile_pool(name="psA1", bufs=6, space="PSUM") as psA1,
            ):
                cos_sb = a1w.tile([64, S], bf16, tag="cos")
                sin_sb = a1w.tile([64, S], bf16, tag="sin")
                wq_sb = a1w.tile([128, NE, J], f32r, tag="wq")
                wk_sb = a1w.tile([128, NE, J], f32r, tag="wk")
                xTr = xT.rearrange("(ne p) s -> p ne s", p=128)
                xc_cache = {}
                xc0 = a1x.tile([128, NE, SC], f32r, tag="xc")
                # interleave weight and first-chunk quarters so the first
                # accumulation chain starts after one quarter of each
                for eq in range(4):
                    nc.sync.dma_start(
                        wq_sb[:, eq * 4:(eq + 1) * 4, :],
                        wqT.rearrange("(ne p) j -> p ne j", p=128)[:, eq * 4:(eq + 1) * 4, :])
                    nc.sync.dma_start(
                        xc0[:, eq * 4:(eq + 1) * 4, :],
                        xTr[:, eq * 4:(eq + 1) * 4, 0:SC])
                    nc.sync.dma_start(
                        wk_sb[:, eq * 4:(eq + 1) * 4, :],
                        wkT.rearrange("(ne p) j -> p ne j", p=128)[:, eq * 4:(eq + 1) * 4, :])
                nc.sync.dma_start(cos_sb[:], cosb[:])
                nc.sync.dma_start(sin_sb[:], sinb[:])
                nc.sync.dma_start(mask_sb[:], maskd[:])
                for sc in range(NSC):
                    if sc == 0:
                        xc = xc0
                    else:
                        xc = a1x.tile([128, NE, SC], f32r, tag="xc")
                        for eq in range(4):
                            nc.sync.dma_start(
                                xc[:, eq * 4:(eq + 1) * 4, :],
                                xTr[:, eq * 4:(eq + 1) * 4, sc * SC:(sc + 1) * SC],
                            )
                    if sc == NSC - 1:
                        xc_cache[sc] = xc
                    c2 = cos_sb[:, sc * SC:(sc + 1) * SC]
                    s2 = sin_sb[:, sc * SC:(sc + 1) * SC]
                    for w_sb, dstT in ((wq_sb, qT), (wk_sb, kT)):
                        for jt in range(HPG):
                            ps = psA1.tile([128, SC], f32, tag="ps_qk")
                            for et in range(NE):
                                nc.tensor.matmul(
                                    ps[:],
                                    w_sb[:, et, jt * 128:(jt + 1) * 128],
                                    xc[:, et, :],
                                    start=(et == 0), stop=(et == NE - 1),
                                )
                            # RoPE: rows 0:64 = r (even feats), 64:128 = i (odd)
                            dst = dstT[:, jt * S + sc * SC: jt * S + (sc + 1) * SC]
                            t1 = ropep.tile([64, SC], bf16, tag="t1")
                            t2 = ropep.tile([64, SC], bf16, tag="t2")
                            nc.vector.tensor_mul(t1[:], ps[:64, :], c2)
                            nc.vector.tensor_mul(t2[:], ps[64:, :], s2)
                            nc.gpsimd.tensor_sub(dst[:64, :], t1[:], t2[:])
                            t3 = ropep.tile([64, SC], bf16, tag="t1")
                            t4 = ropep.tile([64, SC], bf16, tag="t2")
                            nc.vector.tensor_mul(t3[:], ps[:64, :], s2)
                            nc.vector.tensor_mul(t4[:], ps[64:, :], c2)
                            nc.gpsimd.tensor_add(dst[64:, :], t3[:], t4[:])

                # ---- V projection (same scope; wv reuses the wq slot) ----
                vN = qkp.tile([128, NT * J], bf16, tag="vN")   # V natural [t, d]
                wv_sb = a1w.tile([128, NE, J], f32r, tag="wq")
                nc.sync.dma_start(wv_sb[:], wvT.rearrange("(ne p) j -> p ne j", p=128))
                for sc in [NSC - 1] + list(range(NSC - 1)):
                    if sc in xc_cache:
                        xc = xc_cache[sc]
                    else:
                        xc = a1x.tile([128, NE, SC], f32r, tag="xc")
                        for eq in range(4):
                            nc.sync.dma_start(
                                xc[:, eq * 4:(eq + 1) * 4, :],
                                xTr[:, eq * 4:(eq + 1) * 4, sc * SC:(sc + 1) * SC],
                            )
                    for tt in range(SC // 128):
                        ps = psA1.tile([128, J], f32, tag="ps_qk")
                        for et in range(NE):
                            nc.tensor.matmul(
                                ps[:],
                                xc[:, et, tt * 128:(tt + 1) * 128],
                                wv_sb[:, et, :],
                                start=(et == 0), stop=(et == NE - 1),
                            )
                        gt = sc * (SC // 128) + tt
                        nc.scalar.activation(vN[:, gt * J:(gt + 1) * J], ps[:], AF.Copy)

            # =========== Phase B: attention ===========
            with (
                tc.tile_pool(name="oN", bufs=1) as onp,
                tc.tile_pool(name="att", bufs=3) as attp,
                tc.tile_pool(name="psS", bufs=2, space="PSUM") as psS,
                tc.tile_pool(name="psT", bufs=2, space="PSUM") as psT,
                tc.tile_pool(name="psO", bufs=2, space="PSUM") as psO,
            ):
                # O natural: s-tile st -> [:, st*J:(st+1)*J] = [128 s, J hd]
                oN = onp.tile([128, NT * J], f32r, tag="oN")

                for ht in range(HPG):
                    qh = qT[:, ht * S:(ht + 1) * S]
                    kh = kT[:, ht * S:(ht + 1) * S]
                    for qb in range(NT):
                        t_ext = (qb + 1) * 128 if causal else S
                        nkt = t_ext // 128
                        nch = (t_ext + 511) // 512
                        e_sb = attp.tile([128, S], bf16, tag="e_sb")
                        den_parts = attp.tile([128, 4], f32, tag="denp")
                        for ch in range(nch):
                            n = min(512, t_ext - ch * 512)
                            ps = psS.tile([128, 512], f32, tag="ps_s")
                            nc.tensor.matmul(
                                ps[:, :n],
                                qh[:, qb * 128:(qb + 1) * 128],
                                kh[:, ch * 512: ch * 512 + n],
                                start=True, stop=True,
                            )
                            if causal and ch == nch - 1:
                                nc.vector.tensor_add(
                                    ps[:, n - 128:n], ps[:, n - 128:n],
                                    mask_sb[:, qb * 128:(qb + 1) * 128],
                                )
                            nc.scalar.activation(
                                e_sb[:, ch * 512: ch * 512 + n], ps[:, :n], AF.Exp,
                                scale=SCALE, accum_out=den_parts[:, ch:ch + 1],
                            )
                        den = attp.tile([128, 1], f32, tag="den")
                        if nch == 1:
                            rec_src = den_parts[:, 0:1]
                        else:
                            nc.vector.tensor_add(den[:], den_parts[:, 0:1], den_parts[:, 1:2])
                            for ch in range(2, nch):
                                nc.vector.tensor_add(den[:], den[:], den_parts[:, ch:ch + 1])
                            rec_src = den[:]
                        rec = attp.tile([128, 1], f32, tag="rec")
                        nc.vector.reciprocal(rec[:], rec_src)

                        eT_sb = attp.tile([128, NT * 128], bf16, tag="eT")
                        for kt in range(nkt):
                            pt = psT.tile([128, 128], bf16, tag="ps_t")
                            nc.tensor.transpose(pt[:], e_sb[:, kt * 128:(kt + 1) * 128], ident[:])
                            dst = eT_sb[:, kt * 128:(kt + 1) * 128]
                            if kt % 2 == 0:
                                nc.vector.tensor_copy(dst, pt[:])
                            else:
                                nc.scalar.activation(dst, pt[:], AF.Copy)
                        po = psO.tile([128, 128], f32, tag="ps_o")
                        for kt in range(nkt):
                            nc.tensor.matmul(
                                po[:],
                                eT_sb[:, kt * 128:(kt + 1) * 128],
                                vN[:, kt * J + ht * 128: kt * J + (ht + 1) * 128],
                                start=(kt == 0), stop=(kt == nkt - 1),
                            )
                        nc.scalar.activation(
                            oN[:, qb * J + ht * 128: qb * J + (ht + 1) * 128],
                            po[:], AF.Copy, scale=rec[:],
                        )

                # =========== Phase C: output projection ===========
                EC = 512
                if "C" in phases:
                 with (
                    tc.tile_pool(name="wo", bufs=2) as wop,
                    tc.tile_pool(name="psY", bufs=4, space="PSUM") as psY,
                ):
                    for ec in range(DIM // EC):
                        woc = wop.tile([128, NT, EC], f32r, tag="woc")
                        woTr = woT.rearrange("(nt p) e -> p nt e", p=128)
                        for sq in range(4):
                            nc.sync.dma_start(
                                woc[:, sq * 4:(sq + 1) * 4, :],
                                woTr[:, sq * 4:(sq + 1) * 4, ec * EC:(ec + 1) * EC],
                            )
                        for jt in range(HPG):
                            ps = psY.tile([128, EC], f32, tag="ps_y")
                            for st in range(NT):
                                nc.tensor.matmul(
                                    ps[:],
                                    oN[:, st * J + jt * 128: st * J + (jt + 1) * 128],
                                    woc[:, st, :],
                                    start=(st == 0), stop=(st == NT - 1),
                                )
                            ysb = wop.tile([128, EC], f32, tag="ysb")
                            nc.scalar.activation(ysb[:], ps[:], AF.Copy)
                            nc.sync.dma_start(
                                y[jt * 128:(jt + 1) * 128, ec * EC:(ec + 1) * EC], ysb[:]
                            )

    import bass_rust
    bass_rust.move_matmul_waits_to_ldweights(nc.m)
    bass_rust.generate_event_semaphores(nc)
    return nc


def _get_program(causal: bool):
    if causal not in _PROGRAMS:
        _PROGRAMS[causal] = _build_program(causal)
    return _PROGRAMS[causal]


def _deinterleave_rows(w_slice):
    """Permute [128k, E] rows within each 128-row head block: evens then odds."""
    out = w_slice.reshape(-1, DH, w_slice.shape[-1])
    return np.concatenate([out[:, 0::2, :], out[:, 1::2, :]], axis=1).reshape(w_slice.shape)


def _is_causal_compatible(mask2d):
    causal_ref = np.triu(np.full((S, S), -1e9, dtype=np.float32), k=1)
    if np.array_equal(mask2d, causal_ref):
        return True
    # any mask that is 0 on/below the block sub-diagonal region outside the
    # diagonal tiles and <= -1e8 strictly above the diagonal tiles also works
    for i in range(NT):
        lo = mask2d[i * 128:(i + 1) * 128, : i * 128]
        if lo.size and not np.all(lo == 0.0):
            return False
        up = mask2d[i * 128:(i + 1) * 128, (i + 1) * 128:]
        if up.size and not np.all(up <= -1e8):
            return False
    return True


def _make_in_maps(inputs):
    x = np.asarray(inputs["x"], dtype=np.float32)
    Wq = np.asarray(inputs["Wq"], dtype=np.float32)
    Wk = np.asarray(inputs["Wk"], dtype=np.float32)
    Wv = np.asarray(inputs["Wv"], dtype=np.float32)
    Wo = np.asarray(inputs["Wo"], dtype=np.float32)
    freqs_cos = np.asarray(inputs["freqs_cos"], dtype=np.float32)
    freqs_sin = np.asarray(inputs["freqs_sin"], dtype=np.float32)
    mask2d = np.asarray(inputs["mask"], dtype=np.float32).reshape(S, S)

    import ml_dtypes
    cosb = np.ascontiguousarray(freqs_cos.T).astype(ml_dtypes.bfloat16)
    sinb = np.ascontiguousarray(freqs_sin.T).astype(ml_dtypes.bfloat16)
    maskd = np.concatenate(
        [mask2d[i * 128:(i + 1) * 128, i * 128:(i + 1) * 128] for i in range(NT)], axis=1
    ) * np.float32(np.sqrt(DH))
    import ml_dtypes as _mld
    maskd = np.ascontiguousarray(maskd).astype(_mld.bfloat16)
    woT = np.ascontiguousarray(Wo.T)

    in_maps = []
    for c in range(8):
        b, g = divmod(c, G)
        rows = slice(g * J, (g + 1) * J)
        in_maps.append({
            "xT": np.ascontiguousarray(x[b].T),
            "wqT": np.ascontiguousarray(_deinterleave_rows(Wq[rows]).T),
            "wkT": np.ascontiguousarray(_deinterleave_rows(Wk[rows]).T),
            "wvT": np.ascontiguousarray(Wv[rows].T),
            "woT": woT,
            "cosb": cosb,
            "sinb": sinb,
            "maskd": maskd,
        })
    return in_maps


def _offdiag_tiles_zero(mask2d):
    m = mask2d.copy()
    for i in range(NT):
        m[i * 128:(i + 1) * 128, i * 128:(i + 1) * 128] = 0.0
    return bool(np.all(m == 0.0))


def _numpy_fallback(x, Wq, Wk, Wv, Wo, freqs_cos, freqs_sin, mask):
    q = (x @ Wq.T).reshape(B, S, H, DH)
    k = (x @ Wk.T).reshape(B, S, H, DH)
    v = (x @ Wv.T).reshape(B, S, H, DH)

    def rope(t):
        tr, ti = t[..., 0::2], t[..., 1::2]
        c = freqs_cos[None, :, None, :]
        s = freqs_sin[None, :, None, :]
        return np.stack([tr * c - ti * s, tr * s + ti * c], axis=-1).reshape(t.shape)

    q, k = rope(q), rope(k)
    q, k, v = (t.transpose(0, 2, 1, 3) for t in (q, k, v))
    m = mask.reshape(S, S)
    out = np.empty((B, H, S, DH), np.float32)
    for b in range(B):
        for h in range(H):
            sc = (q[b, h] @ k[b, h].T) / np.float32(np.sqrt(DH)) + m
            sc -= sc.max(axis=1, keepdims=True)
            e = np.exp(sc)
            out[b, h] = (e / e.sum(axis=1, keepdims=True)) @ v[b, h]
    out = out.transpose(0, 1, 3, 2).reshape(B, S, DIM)
    return (out @ Wo.T).astype(np.float32)


def kernel(x, Wq, Wk, Wv, Wo, freqs_cos, freqs_sin, mask):
    from concourse.bass_utils import run_bass_kernel_spmd

    inputs = {"x": x, "Wq": Wq, "Wk": Wk, "Wv": Wv, "Wo": Wo,
              "freqs_cos": freqs_cos, "freqs_sin": freqs_sin, "mask": mask}
    mask2d = np.asarray(mask, dtype=np.float32).reshape(S, S)
    causal = _is_causal_compatible(mask2d)
    if not causal and not _offdiag_tiles_zero(mask2d):
        return _numpy_fallback(
            np.asarray(x, np.float32), np.asarray(Wq, np.float32),
            np.asarray(Wk, np.float32), np.asarray(Wv, np.float32),
            np.asarray(Wo, np.float32), np.asarray(freqs_cos, np.float32),
            np.asarray(freqs_sin, np.float32), mask2d)
    nc = _get_program(causal)
    in_maps = _make_in_maps(inputs)

    res = run_bass_kernel_spmd(nc, in_maps, core_ids=list(range(8)))

    out = np.empty((B, S, DIM), dtype=np.float32)
    for c in range(8):
        b, g = divmod(c, G)
        out[b, g * J:(g + 1) * J, :] = res.results[c]["y"]
    return out
